# revision 19
# baseline (speedup 1.0000x reference)
"""Trainium2 Bass kernel for multi-head attention (nn_AttentionWithDropout).

Reference computation (fp32):
    q = query @ Wq.T + bq ; k = key @ Wk.T + bk ; v = value @ Wv.T + bv
    per head: P = softmax(q k^T / sqrt(E)) ; o = P v
    out = concat_heads(o) @ Wo.T + bo

Sharding (8 cores): data-parallel over batch (2 groups of 4 cores) x
tensor-parallel over heads (4 heads / 256 channels per core, Megatron
column-sharded Wq/Wk/Wv).  All matmul operands are bf16 (fp32r matmuls
power-throttle the PE; bf16 runs at 1 cyc/row and halves HBM +
collective traffic).  PSUM accumulation stays fp32.

Per core: K proj -> V proj (chan-major like K, then XBAR DMA-transpose
into the [tok, chan] PV layout) -> per 512-token chunk qc: Q proj,
attention for 4 heads, per-head-pair AllGather of the normalized
attention output [128, 512] within the 4-core batch group, and the full
output projection for the chunk on this core's 256 out-channel slice.

The tensor engine executes its instruction stream in order, so emission
order is the schedule:
  - each head-pair's normalize tail (1/r broadcast matmul, scale,
    ag store) is DEFERRED ~3 kt-steps into the next sweep so the DVE
    reciprocal completes in the shadow of real work instead of stalling
    the PE;
  - out-proj for chunk qc is emitted inside attention(qc+1), by which
    time its AllGathers have long finished;
  - Q proj for chunk qc+1 is emitted right after attention(qc), covering
    the final reciprocal latency.

Softmax skips the max-subtraction: energies are ~N(0, 0.25^2) here, so
exp() is safe; row sums come from an all-ones column appended to V (PV
yields [o | r] in one accumulation).  Normalization multiplies by 1/r:
the two r-rows sit at partitions 0/32 of a memset-to-1 [33,512] tile
(engine writes must start at a 32-aligned partition), one DVE reciprocal
covers both, and a [33,128] selector matmul broadcasts 1/r across the
128 output partitions.
"""

import sys

sys.path.insert(0, "/opt/trn_rl_repo")

import numpy as np
import ml_dtypes

BF16 = ml_dtypes.bfloat16

# ---- problem constants (hardcoded per the harness contract) ----
B, L, E = 2, 2048, 1024
H, D = 16, 64
N_CORES = 8
TP = 4                  # cores per batch group (head-parallel)
CH = E // TP            # 256 channels (4 heads) per core
NQC = 4                 # token chunks
QC = L // NQC           # 512 tokens per chunk
SCALE = 1.0 / 32.0      # 1/sqrt(E)
KT = E // 128           # 8 contraction tiles for projections
NKT = L // 128          # 16 key-token tiles


def _split_multi_waits(nc):
    """The nix walrus in this container only encodes one semaphore wait per
    instruction (setupSyncWait raises "Too many sync wait commands" above
    that).  Tile's wait assignment attaches several.  Hoist the extras into
    standalone InstEventSemaphore waits (the encoding `engine.wait_ge` uses)
    immediately before the owning instruction, preserving per-engine order
    and exact semantics."""
    from concourse import mybir

    n_split = 0
    for fn in nc.m.functions:
        for bb in fn.blocks:
            out = []
            for inst in bb.instructions:
                si = inst.sync_info
                if si is not None and si.on_wait and len(si.on_wait) > 1:
                    waits = list(si.on_wait)
                    for k, w in enumerate(waits[:-1]):
                        wi = mybir.InstEventSemaphore(
                            name=f"{inst.name}-hw{k}", ins=[], outs=[])
                        wi.engine = inst.engine
                        wi.debug = inst.debug
                        wi.sync_info = mybir.SyncInfo(on_wait=[w],
                                                      on_update=[])
                        out.append(wi)
                        n_split += 1
                    si.on_wait = [waits[-1]]
                out.append(inst)
            bb.instructions[:] = out
    return n_split


def _build_nc():
    import concourse.bass as bass
    import concourse.tile as tile
    from concourse import masks, mybir

    f32 = mybir.dt.float32
    bf16 = mybir.dt.bfloat16
    AF = mybir.ActivationFunctionType

    nc = bass.Bass("TRN2", target_bir_lowering=False, debug=False,
                   num_devices=N_CORES)

    # ---- per-core external IO ----
    xqT = nc.dram_tensor("xqT", [E, L], bf16, kind="ExternalInput")
    xkT = nc.dram_tensor("xkT", [E, L], bf16, kind="ExternalInput")
    xvT = nc.dram_tensor("xvT", [E, L], bf16, kind="ExternalInput")
    wqT = nc.dram_tensor("wqT", [E, CH], bf16, kind="ExternalInput")
    wkT = nc.dram_tensor("wkT", [E, CH], bf16, kind="ExternalInput")
    wvT = nc.dram_tensor("wvT", [E, CH], bf16, kind="ExternalInput")
    bqc = nc.dram_tensor("bqc", [CH], f32, kind="ExternalInput")
    bkc = nc.dram_tensor("bkc", [CH], f32, kind="ExternalInput")
    bvc = nc.dram_tensor("bvc", [CH], f32, kind="ExternalInput")
    woT = nc.dram_tensor("woT", [E, CH], bf16, kind="ExternalInput")
    bor = nc.dram_tensor("bor", [128, CH], f32, kind="ExternalInput")
    sel2 = nc.dram_tensor("sel2", [33, 128], bf16, kind="ExternalInput")
    out = nc.dram_tensor("out", [L, CH], f32, kind="ExternalOutput")

    with tile.TileContext(nc) as tc:
        with (
            tc.tile_pool(name="consts", bufs=1) as consts,
            tc.tile_pool(name="persist", bufs=1) as persist,
            tc.tile_pool(name="dram", bufs=1, space="DRAM") as dpool,
        ):
            # per (chunk, head-pair) all-gather buffers
            ag_in = [[dpool.tile([128, QC], bf16, name=f"agi{i}_{h}")
                      for h in range(2)] for i in range(NQC)]
            ag_out = [[dpool.tile([TP, 128, QC], bf16, name=f"ago{i}_{h}")
                       for h in range(2)] for i in range(NQC)]

            # ---- input loads first (DMA queue order = program order) ----
            wk_sb = [persist.tile([128, CH], bf16, name=f"wk{i}")
                     for i in range(KT)]
            for i in range(KT):
                nc.sync.dma_start(wk_sb[i][:], wkT[i * 128:(i + 1) * 128, :])
            xk = [persist.tile([128, L], bf16, name=f"xk{i}")
                  for i in range(KT)]
            for i in range(KT):
                nc.sync.dma_start(xk[i][:], xkT[i * 128:(i + 1) * 128, :])

            bias_cols = {}
            for nm, src in (("q", bqc), ("k", bkc), ("v", bvc)):
                for ct in range(CH // 128):
                    t = consts.tile([128, 1], f32, name=f"b{nm}{ct}")
                    nc.sync.dma_start(
                        t[:], src[ct * 128:(ct + 1) * 128].unsqueeze(1))
                    bias_cols[(nm, ct)] = t

            wv_sb = [persist.tile([128, CH], bf16, name=f"wv{i}")
                     for i in range(KT)]
            for i in range(KT):
                nc.sync.dma_start(wv_sb[i][:], wvT[i * 128:(i + 1) * 128, :])
            xv = [persist.tile([128, L], bf16, name=f"xv{i}")
                  for i in range(KT)]
            for i in range(KT):
                nc.sync.dma_start(xv[i][:], xvT[i * 128:(i + 1) * 128, :])
            wq_sb = [persist.tile([128, CH], bf16, name=f"wq{i}")
                     for i in range(KT)]
            for i in range(KT):
                nc.sync.dma_start(wq_sb[i][:], wqT[i * 128:(i + 1) * 128, :])
            xq = [persist.tile([128, L], bf16, name=f"xq{i}")
                  for i in range(KT)]
            for i in range(KT):
                nc.sync.dma_start(xq[i][:], xqT[i * 128:(i + 1) * 128, :])

            ones2 = consts.tile([33, 128], bf16)
            nc.sync.dma_start(ones2[:], sel2[:])
            ident = consts.tile([128, 128], bf16)
            masks.make_identity(nc, ident[:])
            bo_bcast = consts.tile([128, CH], f32)
            nc.sync.dma_start(bo_bcast[:], bor[:])
            woT_sb = [persist.tile([128, CH], bf16, name=f"woT{i}")
                      for i in range(KT)]
            for i in range(KT):
                nc.sync.dma_start(woT_sb[i][:], woT[i * 128:(i + 1) * 128, :])

            # ---- persistent SBUF tensors ----
            qT = [persist.tile([128, L], bf16, name=f"qT{i}")
                  for i in range(2)]
            kTt = [persist.tile([128, L], bf16, name=f"kT{i}")
                   for i in range(2)]
            v_cm = [persist.tile([128, L], bf16, name=f"vc{i}")
                    for i in range(2)]
            # v tiles: [tok 128, 4 heads x (64 v-cols + 1 ones-col)]
            v_sb = [persist.tile([128, 4, 65], bf16, name=f"v{t}")
                    for t in range(NKT)]
            for t in range(NKT):
                nc.vector.memset(v_sb[t][:, :, 64:65], 1.0)

            # ================= K / V projections =================
            # chan-major: proj[ct][:, t] = (W x)[ct-chans, t] + b
            with (
                tc.tile_pool(name="kps", bufs=4, space="PSUM") as kps,
                tc.tile_pool(name="pvt", bufs=2, space="PSUM") as pvt,
            ):
                for nm, w_sb, x_sb, dst in (("k", wk_sb, xk, kTt),
                                            ("v", wv_sb, xv, v_cm)):
                    for ct in range(2):
                        pss = [kps.tile([128, QC], f32, name="kps")
                               for _ in range(NQC)]
                        for kt in range(KT):
                            for t5 in range(NQC):
                                nc.tensor.matmul(
                                    pss[t5][:],
                                    w_sb[kt][:, ct * 128:(ct + 1) * 128],
                                    x_sb[kt][:, t5 * QC:(t5 + 1) * QC],
                                    start=(kt == 0), stop=(kt == KT - 1))
                        for t5 in range(NQC):
                            nc.vector.tensor_scalar_add(
                                dst[ct][:, t5 * QC:(t5 + 1) * QC],
                                pss[t5][:], bias_cols[(nm, ct)][:])
                # transpose V into the [tok, chan] PV layout:
                # v_cm[ct][c, tt*128 + t] -> v_sb[tt][t, 2*ct + c//64, c%64]
                for tt in range(NKT):
                    for ct in range(2):
                        pt = pvt.tile([128, 128], bf16, name="pt")
                        nc.tensor.transpose(
                            pt[:], v_cm[ct][:, tt * 128:(tt + 1) * 128],
                            ident[:])
                        nc.vector.tensor_copy(
                            v_sb[tt][:, 2 * ct:2 * ct + 2, 0:64],
                            pt.rearrange("p (h d) -> p h d", h=2))

            # ========== per-chunk: Q proj, attention, AG, out proj ==========
            with (
                tc.tile_pool(name="pst", bufs=2, space="PSUM") as pst,
                tc.tile_pool(name="pacc", bufs=2, space="PSUM") as pacc,
                tc.tile_pool(name="pout", bufs=2, space="PSUM") as pout,
                tc.tile_pool(name="upool", bufs=4) as upool,
                tc.tile_pool(name="aop", bufs=2) as aop,
                tc.tile_pool(name="rp", bufs=2) as rp,
                tc.tile_pool(name="rip", bufs=2) as rip,
                tc.tile_pool(name="aogp", bufs=2) as aogp,
                tc.tile_pool(name="apool", bufs=16) as apool,
                tc.tile_pool(name="obp", bufs=2) as obp,
            ):
                # deferred normalize tails; flushed a few kt-steps into the
                # next sweep so bc never stalls the PE on the reciprocal
                pending = []

                def flush_pending():
                    for fn in pending:
                        fn()
                    pending.clear()

                def qproj(qc):
                    # Q projection for chunk qc (qps rides the st ring)
                    qps = pst.tile([128, 1024], f32, name="st", tag="st")
                    for ct in range(2):
                        for kt in range(KT):
                            nc.tensor.matmul(
                                qps[:, ct * QC:(ct + 1) * QC],
                                wq_sb[kt][:, ct * 128:(ct + 1) * 128],
                                xq[kt][:, qc * QC:(qc + 1) * QC],
                                start=(kt == 0), stop=(kt == KT - 1))
                        nc.vector.tensor_scalar_add(
                            qT[ct][:, qc * QC:(qc + 1) * QC],
                            qps[:, ct * QC:(ct + 1) * QC],
                            bias_cols[("q", ct)][:])

                def norm_tail(qc, hp, accs, rinv):
                    def emit():
                        bc = pst.tile([128, QC], f32, name="bc", tag="st",
                                      padded_shape=[128, 1024])
                        nc.tensor.matmul(bc[:], ones2[:], rinv[:],
                                         start=True, stop=True)
                        aog = aogp.tile([128, QC], bf16, name="aog")
                        nc.vector.tensor_mul(aog[:], accs[:], bc[:])
                        nc.sync.dma_start(ag_in[qc][hp][:], aog[:])
                        nc.gpsimd.collective_compute(
                            "AllGather", mybir.AluOpType.bypass,
                            replica_groups=[[0, 1, 2, 3], [4, 5, 6, 7]],
                            ins=[ag_in[qc][hp].opt()],
                            outs=[ag_out[qc][hp].opt()])
                    return emit

                def outproj(qc):
                    # full output projection for chunk qc's 512 tokens;
                    # emitted inside attention(qc+1), when AG(qc) is done
                    ach = []
                    for kt8 in range(KT):
                        src, hp = divmod(kt8, 2)
                        a = apool.tile([128, QC], bf16, name="a")
                        nc.sync.dma_start(a[:], ag_out[qc][hp][src, :, :])
                        ach.append(a)
                    for ti in range(4):
                        po = pout.tile([128, CH], f32, name="po")
                        for kt8 in range(KT):
                            nc.tensor.matmul(
                                po[:],
                                ach[kt8][:, ti * 128:(ti + 1) * 128],
                                woT_sb[kt8][:],
                                start=(kt8 == 0), stop=(kt8 == KT - 1))
                        ob = obp.tile([128, CH], f32, name="ob")
                        nc.vector.tensor_add(ob[:], po[:], bo_bcast[:])
                        row0 = qc * QC + ti * 128
                        nc.sync.dma_start(out[row0:row0 + 128, :], ob[:])

                qproj(0)
                for qc in range(NQC):
                    for hp in range(2):
                        accs = [pacc.tile([65, QC], f32, name="acc")
                                for _ in range(2)]
                        rsb = rp.tile([33, QC], f32, name="rsb")
                        nc.vector.memset(rsb[:], 1.0)
                        for kt in range(NKT):
                            if kt == 3:
                                flush_pending()
                            st = pst.tile([128, 1024], f32, name="st",
                                          tag="st")
                            for j in range(2):
                                nc.tensor.matmul(
                                    st[:, j * QC:(j + 1) * QC],
                                    kTt[hp][j * 64:(j + 1) * 64,
                                            kt * 128:(kt + 1) * 128],
                                    qT[hp][j * 64:(j + 1) * 64,
                                           qc * QC:(qc + 1) * QC],
                                    start=True, stop=True)
                            u = upool.tile([128, 1024], bf16, name="u")
                            nc.scalar.activation(u[:], st[:], AF.Exp,
                                                 scale=SCALE)
                            for j in range(2):
                                nc.tensor.matmul(
                                    accs[j][:],
                                    v_sb[kt][:, 2 * hp + j, :],
                                    u[:, j * QC:(j + 1) * QC],
                                    start=(kt == 0), stop=(kt == NKT - 1))
                        # normalize, part 1: drain accs, one reciprocal
                        aosb = aop.tile([128, QC], bf16, name="aosb")
                        for j in range(2):
                            nc.vector.tensor_copy(rsb[32 * j:32 * j + 1, :],
                                                  accs[j][64:65, :])
                            nc.vector.tensor_copy(aosb[j * 64:(j + 1) * 64, :],
                                                  accs[j][0:64, :])
                        rinv = rip.tile([33, QC], bf16, name="rinv")
                        with nc.allow_low_precision("softmax denom in bf16"):
                            nc.vector.reciprocal(rinv[:], rsb[:])
                        tail = norm_tail(qc, hp, aosb, rinv)
                        if hp == 0:
                            pending.append(tail)
                        elif qc + 1 < NQC:
                            # cover the reciprocal latency with Q proj of
                            # the next chunk, then emit the tail inline
                            qproj(qc + 1)
                            tail()
                            if qc > 0:
                                outproj(qc - 1)
                        else:
                            # last chunk: out-proj of the previous chunk
                            # covers the reciprocal instead
                            outproj(qc - 1)
                            tail()
                flush_pending()
                outproj(NQC - 1)

    _split_multi_waits(nc)
    return nc


def _sel2():
    s = np.zeros((33, 128), dtype=BF16)
    s[0, 0:64] = 1
    s[32, 64:128] = 1
    return s


_NC_CACHE = {}


def _get_nc():
    if "nc" not in _NC_CACHE:
        _NC_CACHE["nc"] = _build_nc()
    return _NC_CACHE["nc"]


def kernel(query, key, value, Wq, bq, Wk, bk, Wv, bv, Wo, bo,
           _trace=False, _trace_cores=None):
    from concourse.bass_utils import run_bass_kernel_spmd

    query = np.asarray(query, dtype=np.float32)
    key = np.asarray(key, dtype=np.float32)
    value = np.asarray(value, dtype=np.float32)
    Wq = np.asarray(Wq, dtype=np.float32)
    bq = np.asarray(bq, dtype=np.float32)
    Wk = np.asarray(Wk, dtype=np.float32)
    bk = np.asarray(bk, dtype=np.float32)
    Wv = np.asarray(Wv, dtype=np.float32)
    bv = np.asarray(bv, dtype=np.float32)
    Wo = np.asarray(Wo, dtype=np.float32)
    bo = np.asarray(bo, dtype=np.float32)

    nc = _get_nc()

    xT = {b: {"q": np.ascontiguousarray(query[b].T).astype(BF16),
              "k": np.ascontiguousarray(key[b].T).astype(BF16),
              "v": np.ascontiguousarray(value[b].T).astype(BF16)}
          for b in range(B)}

    in_maps = []
    for c in range(N_CORES):
        b, g = divmod(c, TP)
        sl = slice(g * CH, (g + 1) * CH)
        in_maps.append({
            "xqT": xT[b]["q"], "xkT": xT[b]["k"], "xvT": xT[b]["v"],
            "wqT": np.ascontiguousarray(Wq[sl, :].T).astype(BF16),
            "wkT": np.ascontiguousarray(Wk[sl, :].T).astype(BF16),
            "wvT": np.ascontiguousarray(Wv[sl, :].T).astype(BF16),
            "bqc": bq[sl], "bkc": bk[sl], "bvc": bv[sl],
            "woT": np.ascontiguousarray(Wo[sl, :].T).astype(BF16),
            "bor": np.ascontiguousarray(
                np.broadcast_to(bo[sl].reshape(1, CH), (128, CH))),
            "sel2": _sel2(),
        })

    kwargs = {}
    if _trace:
        kwargs.update(trace=True,
                      trace_cores=_trace_cores or list(range(N_CORES)))
    res = run_bass_kernel_spmd(nc, in_maps, core_ids=list(range(N_CORES)),
                               **kwargs)

    full = np.empty((B, L, E), dtype=np.float32)
    for c in range(N_CORES):
        b, g = divmod(c, TP)
        full[b, :, g * CH:(g + 1) * CH] = res.results[c]["out"]

    if _trace:
        kernel.last_exec_ns = res.exec_time_ns
        kernel.last_results = res
    return full


# revision 23
# speedup vs baseline: 1.1574x; 1.1574x over previous
"""Trainium2 Bass kernel for multi-head attention (nn_AttentionWithDropout).

Reference computation (fp32):
    q = query @ Wq.T + bq ; k = key @ Wk.T + bk ; v = value @ Wv.T + bv
    per head: P = softmax(q k^T / sqrt(E)) ; o = P v
    out = concat_heads(o) @ Wo.T + bo

Sharding (8 cores): data-parallel over batch (2 groups of 4 cores) x
tensor-parallel over heads (4 heads / 256 channels per core, Megatron
column-sharded Wq/Wk/Wv).  All matmul operands are bf16 (fp32r matmuls
power-throttle the PE; bf16 runs at 1 cyc/row and halves HBM +
collective traffic).  PSUM accumulation stays fp32.

Per core: K proj -> V proj (chan-major like K, then XBAR DMA-transpose
into the [tok, chan] PV layout) -> per 512-token chunk qc: Q proj,
attention for 4 heads, per-head-pair AllGather of the normalized
attention output [128, 512] within the 4-core batch group, and the full
output projection for the chunk on this core's 256 out-channel slice.

The tensor engine executes its instruction stream in order, so emission
order is the schedule:
  - each head-pair's normalize tail (1/r broadcast matmul, scale,
    ag store) is DEFERRED ~3 kt-steps into the next sweep so the DVE
    reciprocal completes in the shadow of real work instead of stalling
    the PE;
  - out-proj for chunk qc is emitted inside attention(qc+1), by which
    time its AllGathers have long finished;
  - Q proj for chunk qc+1 is emitted right after attention(qc), covering
    the final reciprocal latency.

Softmax skips the max-subtraction: energies are ~N(0, 0.25^2) here, so
exp() is safe; row sums come from an all-ones column appended to V (PV
yields [o | r] in one accumulation).  Normalization multiplies by 1/r:
the two r-rows sit at partitions 0/32 of a memset-to-1 [33,512] tile
(engine writes must start at a 32-aligned partition), one DVE reciprocal
covers both, and a [33,128] selector matmul broadcasts 1/r across the
128 output partitions.
"""

import sys

sys.path.insert(0, "/opt/trn_rl_repo")

import os

import numpy as np
import ml_dtypes

BF16 = ml_dtypes.bfloat16

# 1 = one AllGather per (chunk, head-pair) [8 ops], 0 = one per chunk [4]
AG_HP_SPLIT = os.environ.get("KERNEL_AG_HP_SPLIT", "0") == "1"

# ---- problem constants (hardcoded per the harness contract) ----
B, L, E = 2, 2048, 1024
H, D = 16, 64
N_CORES = 8
TP = 4                  # cores per batch group (head-parallel)
CH = E // TP            # 256 channels (4 heads) per core
NQC = 4                 # token chunks
QC = L // NQC           # 512 tokens per chunk
SCALE = 1.0 / 32.0      # 1/sqrt(E)
KT = E // 128           # 8 contraction tiles for projections
NKT = L // 128          # 16 key-token tiles


def _split_multi_waits(nc):
    """The nix walrus in this container only encodes one semaphore wait per
    instruction (setupSyncWait raises "Too many sync wait commands" above
    that).  Tile's wait assignment attaches several.  Hoist the extras into
    standalone InstEventSemaphore waits (the encoding `engine.wait_ge` uses)
    immediately before the owning instruction, preserving per-engine order
    and exact semantics."""
    from concourse import mybir

    n_split = 0
    for fn in nc.m.functions:
        for bb in fn.blocks:
            out = []
            for inst in bb.instructions:
                si = inst.sync_info
                if si is not None and si.on_wait and len(si.on_wait) > 1:
                    waits = list(si.on_wait)
                    for k, w in enumerate(waits[:-1]):
                        wi = mybir.InstEventSemaphore(
                            name=f"{inst.name}-hw{k}", ins=[], outs=[])
                        wi.engine = inst.engine
                        wi.debug = inst.debug
                        wi.sync_info = mybir.SyncInfo(on_wait=[w],
                                                      on_update=[])
                        out.append(wi)
                        n_split += 1
                    si.on_wait = [waits[-1]]
                out.append(inst)
            bb.instructions[:] = out
    return n_split


def _build_nc():
    import concourse.bass as bass
    import concourse.tile as tile
    from concourse import masks, mybir

    f32 = mybir.dt.float32
    bf16 = mybir.dt.bfloat16
    AF = mybir.ActivationFunctionType

    nc = bass.Bass("TRN2", target_bir_lowering=False, debug=False,
                   num_devices=N_CORES)

    # ---- per-core external IO ----
    xqT = nc.dram_tensor("xqT", [E, L], bf16, kind="ExternalInput")
    xkT = nc.dram_tensor("xkT", [E, L], bf16, kind="ExternalInput")
    xvT = nc.dram_tensor("xvT", [E, L], bf16, kind="ExternalInput")
    wqT = nc.dram_tensor("wqT", [E, CH], bf16, kind="ExternalInput")
    wkT = nc.dram_tensor("wkT", [E, CH], bf16, kind="ExternalInput")
    wvT = nc.dram_tensor("wvT", [E, CH], bf16, kind="ExternalInput")
    bqc = nc.dram_tensor("bqc", [CH], f32, kind="ExternalInput")
    bkc = nc.dram_tensor("bkc", [CH], f32, kind="ExternalInput")
    bvc = nc.dram_tensor("bvc", [CH], f32, kind="ExternalInput")
    woT = nc.dram_tensor("woT", [E, CH], bf16, kind="ExternalInput")
    bor = nc.dram_tensor("bor", [128, CH], f32, kind="ExternalInput")
    sel2 = nc.dram_tensor("sel2", [33, 128], bf16, kind="ExternalInput")
    out = nc.dram_tensor("out", [L, CH], f32, kind="ExternalOutput")

    with tile.TileContext(nc) as tc:
        with (
            tc.tile_pool(name="consts", bufs=1) as consts,
            tc.tile_pool(name="persist", bufs=1) as persist,
            tc.tile_pool(name="dram", bufs=1, space="DRAM") as dpool,
        ):
            # all-gather buffers: per (chunk, head-pair) when split, else
            # per chunk with both head-pairs stacked [CH, QC]
            if AG_HP_SPLIT:
                ag_in = [[dpool.tile([128, QC], bf16, name=f"agi{i}_{h}")
                          for h in range(2)] for i in range(NQC)]
                ag_out = [[dpool.tile([TP, 128, QC], bf16,
                                      name=f"ago{i}_{h}")
                           for h in range(2)] for i in range(NQC)]
            else:
                ag_in = [dpool.tile([CH, QC], bf16, name=f"agi{i}")
                         for i in range(NQC)]
                ag_out = [dpool.tile([TP, CH, QC], bf16, name=f"ago{i}")
                          for i in range(NQC)]

            # ---- input loads first (DMA queue order = program order) ----
            wk_sb = [persist.tile([128, CH], bf16, name=f"wk{i}")
                     for i in range(KT)]
            for i in range(KT):
                nc.sync.dma_start(wk_sb[i][:], wkT[i * 128:(i + 1) * 128, :])
            xk = [persist.tile([128, L], bf16, name=f"xk{i}")
                  for i in range(KT)]
            for i in range(KT):
                nc.sync.dma_start(xk[i][:], xkT[i * 128:(i + 1) * 128, :])

            bias_cols = {}
            for nm, src in (("q", bqc), ("k", bkc), ("v", bvc)):
                for ct in range(CH // 128):
                    t = consts.tile([128, 1], f32, name=f"b{nm}{ct}")
                    nc.sync.dma_start(
                        t[:], src[ct * 128:(ct + 1) * 128].unsqueeze(1))
                    bias_cols[(nm, ct)] = t

            wv_sb = [persist.tile([128, CH], bf16, name=f"wv{i}")
                     for i in range(KT)]
            for i in range(KT):
                nc.sync.dma_start(wv_sb[i][:], wvT[i * 128:(i + 1) * 128, :])
            xv = [persist.tile([128, L], bf16, name=f"xv{i}")
                  for i in range(KT)]
            for i in range(KT):
                nc.sync.dma_start(xv[i][:], xvT[i * 128:(i + 1) * 128, :])
            wq_sb = [persist.tile([128, CH], bf16, name=f"wq{i}")
                     for i in range(KT)]
            for i in range(KT):
                nc.sync.dma_start(wq_sb[i][:], wqT[i * 128:(i + 1) * 128, :])
            xq = [persist.tile([128, L], bf16, name=f"xq{i}")
                  for i in range(KT)]
            for i in range(KT):
                nc.sync.dma_start(xq[i][:], xqT[i * 128:(i + 1) * 128, :])

            ones2 = consts.tile([33, 128], bf16)
            nc.sync.dma_start(ones2[:], sel2[:])
            ident = consts.tile([128, 128], bf16)
            masks.make_identity(nc, ident[:])
            bo_bcast = consts.tile([128, CH], f32)
            nc.sync.dma_start(bo_bcast[:], bor[:])
            woT_sb = [persist.tile([128, CH], bf16, name=f"woT{i}")
                      for i in range(KT)]
            for i in range(KT):
                nc.sync.dma_start(woT_sb[i][:], woT[i * 128:(i + 1) * 128, :])

            # ---- persistent SBUF tensors ----
            qT = [persist.tile([128, L], bf16, name=f"qT{i}")
                  for i in range(2)]
            kTt = [persist.tile([128, L], bf16, name=f"kT{i}")
                   for i in range(2)]
            v_cm = [persist.tile([128, L], bf16, name=f"vc{i}")
                    for i in range(2)]
            # v tiles: [tok 128, 4 heads x (64 v-cols + 1 ones-col)]
            v_sb = [persist.tile([128, 4, 65], bf16, name=f"v{t}")
                    for t in range(NKT)]
            for t in range(NKT):
                nc.vector.memset(v_sb[t][:, :, 64:65], 1.0)

            # ================= K / V projections =================
            # chan-major: proj[ct][:, t] = (W x)[ct-chans, t] + b
            with (
                tc.tile_pool(name="kps", bufs=4, space="PSUM") as kps,
                tc.tile_pool(name="pvt", bufs=2, space="PSUM") as pvt,
            ):
                for nm, w_sb, x_sb, dst in (("k", wk_sb, xk, kTt),
                                            ("v", wv_sb, xv, v_cm)):
                    for ct in range(2):
                        pss = [kps.tile([128, QC], f32, name="kps")
                               for _ in range(NQC)]
                        for kt in range(KT):
                            for t5 in range(NQC):
                                nc.tensor.matmul(
                                    pss[t5][:],
                                    w_sb[kt][:, ct * 128:(ct + 1) * 128],
                                    x_sb[kt][:, t5 * QC:(t5 + 1) * QC],
                                    start=(kt == 0), stop=(kt == KT - 1))
                        for t5 in range(NQC):
                            nc.vector.tensor_scalar_add(
                                dst[ct][:, t5 * QC:(t5 + 1) * QC],
                                pss[t5][:], bias_cols[(nm, ct)][:])
                # transpose V into the [tok, chan] PV layout:
                # v_cm[ct][c, tt*128 + t] -> v_sb[tt][t, 2*ct + c//64, c%64]
                for tt in range(NKT):
                    for ct in range(2):
                        pt = pvt.tile([128, 128], bf16, name="pt")
                        nc.tensor.transpose(
                            pt[:], v_cm[ct][:, tt * 128:(tt + 1) * 128],
                            ident[:])
                        nc.vector.tensor_copy(
                            v_sb[tt][:, 2 * ct:2 * ct + 2, 0:64],
                            pt.rearrange("p (h d) -> p h d", h=2))

            # ========== per-chunk: Q proj, attention, AG, out proj ==========
            with (
                tc.tile_pool(name="pst", bufs=2, space="PSUM") as pst,
                tc.tile_pool(name="pacc", bufs=2, space="PSUM") as pacc,
                tc.tile_pool(name="pout", bufs=2, space="PSUM") as pout,
                tc.tile_pool(name="upool", bufs=4) as upool,
                tc.tile_pool(name="aop", bufs=2) as aop,
                tc.tile_pool(name="rp", bufs=2) as rp,
                tc.tile_pool(name="rip", bufs=2) as rip,
                tc.tile_pool(name="aogp", bufs=2) as aogp,
                tc.tile_pool(name="apool", bufs=16) as apool,
                tc.tile_pool(name="obp", bufs=2) as obp,
            ):
                # deferred normalize tails; flushed a few kt-steps into the
                # next sweep so bc never stalls the PE on the reciprocal
                pending = []

                def flush_pending():
                    for fn in pending:
                        fn()
                    pending.clear()

                def qproj(qc):
                    # Q projection for chunk qc (qps rides the st ring)
                    qps = pst.tile([128, 1024], f32, name="st", tag="st")
                    for ct in range(2):
                        for kt in range(KT):
                            nc.tensor.matmul(
                                qps[:, ct * QC:(ct + 1) * QC],
                                wq_sb[kt][:, ct * 128:(ct + 1) * 128],
                                xq[kt][:, qc * QC:(qc + 1) * QC],
                                start=(kt == 0), stop=(kt == KT - 1))
                        nc.vector.tensor_scalar_add(
                            qT[ct][:, qc * QC:(qc + 1) * QC],
                            qps[:, ct * QC:(ct + 1) * QC],
                            bias_cols[("q", ct)][:])

                def norm_tail(qc, hp, accs, rinv):
                    def emit():
                        bc = pst.tile([128, QC], f32, name="bc", tag="st",
                                      padded_shape=[128, 1024])
                        nc.tensor.matmul(bc[:], ones2[:], rinv[:],
                                         start=True, stop=True)
                        aog = aogp.tile([128, QC], bf16, name="aog")
                        nc.vector.tensor_mul(aog[:], accs[:], bc[:])
                        if AG_HP_SPLIT:
                            nc.sync.dma_start(ag_in[qc][hp][:], aog[:])
                            nc.gpsimd.collective_compute(
                                "AllGather", mybir.AluOpType.bypass,
                                replica_groups=[[0, 1, 2, 3], [4, 5, 6, 7]],
                                ins=[ag_in[qc][hp].opt()],
                                outs=[ag_out[qc][hp].opt()])
                        else:
                            nc.sync.dma_start(
                                ag_in[qc][hp * 128:(hp + 1) * 128, :],
                                aog[:])
                            if hp == 1:
                                nc.gpsimd.collective_compute(
                                    "AllGather", mybir.AluOpType.bypass,
                                    replica_groups=[[0, 1, 2, 3],
                                                    [4, 5, 6, 7]],
                                    ins=[ag_in[qc].opt()],
                                    outs=[ag_out[qc].opt()])
                    return emit

                def outproj(qc):
                    # full output projection for chunk qc's 512 tokens;
                    # emitted inside attention(qc+1), when AG(qc) is done
                    ach = []
                    for kt8 in range(KT):
                        src, hp = divmod(kt8, 2)
                        a = apool.tile([128, QC], bf16, name="a")
                        if AG_HP_SPLIT:
                            nc.sync.dma_start(a[:],
                                              ag_out[qc][hp][src, :, :])
                        else:
                            nc.sync.dma_start(
                                a[:],
                                ag_out[qc][src,
                                           hp * 128:(hp + 1) * 128, :])
                        ach.append(a)
                    for ti in range(4):
                        po = pout.tile([128, CH], f32, name="po")
                        for kt8 in range(KT):
                            nc.tensor.matmul(
                                po[:],
                                ach[kt8][:, ti * 128:(ti + 1) * 128],
                                woT_sb[kt8][:],
                                start=(kt8 == 0), stop=(kt8 == KT - 1))
                        ob = obp.tile([128, CH], f32, name="ob")
                        nc.vector.tensor_add(ob[:], po[:], bo_bcast[:])
                        row0 = qc * QC + ti * 128
                        nc.sync.dma_start(out[row0:row0 + 128, :], ob[:])

                qproj(0)
                for qc in range(NQC):
                    for hp in range(2):
                        accs = [pacc.tile([65, QC], f32, name="acc")
                                for _ in range(2)]
                        rsb = rp.tile([33, QC], f32, name="rsb")
                        nc.vector.memset(rsb[:], 1.0)
                        for kt in range(NKT):
                            if kt == 3:
                                flush_pending()
                            st = pst.tile([128, 1024], f32, name="st",
                                          tag="st")
                            for j in range(2):
                                nc.tensor.matmul(
                                    st[:, j * QC:(j + 1) * QC],
                                    kTt[hp][j * 64:(j + 1) * 64,
                                            kt * 128:(kt + 1) * 128],
                                    qT[hp][j * 64:(j + 1) * 64,
                                           qc * QC:(qc + 1) * QC],
                                    start=True, stop=True)
                            u = upool.tile([128, 1024], bf16, name="u")
                            nc.scalar.activation(u[:], st[:], AF.Exp,
                                                 scale=SCALE)
                            for j in range(2):
                                nc.tensor.matmul(
                                    accs[j][:],
                                    v_sb[kt][:, 2 * hp + j, :],
                                    u[:, j * QC:(j + 1) * QC],
                                    start=(kt == 0), stop=(kt == NKT - 1))
                        # normalize, part 1: drain accs, one reciprocal
                        aosb = aop.tile([128, QC], bf16, name="aosb")
                        for j in range(2):
                            nc.vector.tensor_copy(rsb[32 * j:32 * j + 1, :],
                                                  accs[j][64:65, :])
                            nc.vector.tensor_copy(aosb[j * 64:(j + 1) * 64, :],
                                                  accs[j][0:64, :])
                        rinv = rip.tile([33, QC], bf16, name="rinv")
                        with nc.allow_low_precision("softmax denom in bf16"):
                            nc.vector.reciprocal(rinv[:], rsb[:])
                        tail = norm_tail(qc, hp, aosb, rinv)
                        if hp == 0:
                            pending.append(tail)
                        elif qc + 1 < NQC:
                            # cover the reciprocal latency with Q proj of
                            # the next chunk, then emit the tail inline
                            qproj(qc + 1)
                            tail()
                            if qc > 0:
                                outproj(qc - 1)
                        else:
                            # last chunk: out-proj of the previous chunk
                            # covers the reciprocal instead
                            outproj(qc - 1)
                            tail()
                flush_pending()
                outproj(NQC - 1)

    _split_multi_waits(nc)
    return nc


def _sel2():
    s = np.zeros((33, 128), dtype=BF16)
    s[0, 0:64] = 1
    s[32, 64:128] = 1
    return s


_NC_CACHE = {}


def _get_nc():
    if "nc" not in _NC_CACHE:
        _NC_CACHE["nc"] = _build_nc()
    return _NC_CACHE["nc"]


def kernel(query, key, value, Wq, bq, Wk, bk, Wv, bv, Wo, bo,
           _trace=False, _trace_cores=None):
    from concourse.bass_utils import run_bass_kernel_spmd

    query = np.asarray(query, dtype=np.float32)
    key = np.asarray(key, dtype=np.float32)
    value = np.asarray(value, dtype=np.float32)
    Wq = np.asarray(Wq, dtype=np.float32)
    bq = np.asarray(bq, dtype=np.float32)
    Wk = np.asarray(Wk, dtype=np.float32)
    bk = np.asarray(bk, dtype=np.float32)
    Wv = np.asarray(Wv, dtype=np.float32)
    bv = np.asarray(bv, dtype=np.float32)
    Wo = np.asarray(Wo, dtype=np.float32)
    bo = np.asarray(bo, dtype=np.float32)

    nc = _get_nc()

    xT = {b: {"q": np.ascontiguousarray(query[b].T).astype(BF16),
              "k": np.ascontiguousarray(key[b].T).astype(BF16),
              "v": np.ascontiguousarray(value[b].T).astype(BF16)}
          for b in range(B)}

    in_maps = []
    for c in range(N_CORES):
        b, g = divmod(c, TP)
        sl = slice(g * CH, (g + 1) * CH)
        in_maps.append({
            "xqT": xT[b]["q"], "xkT": xT[b]["k"], "xvT": xT[b]["v"],
            "wqT": np.ascontiguousarray(Wq[sl, :].T).astype(BF16),
            "wkT": np.ascontiguousarray(Wk[sl, :].T).astype(BF16),
            "wvT": np.ascontiguousarray(Wv[sl, :].T).astype(BF16),
            "bqc": bq[sl], "bkc": bk[sl], "bvc": bv[sl],
            "woT": np.ascontiguousarray(Wo[sl, :].T).astype(BF16),
            "bor": np.ascontiguousarray(
                np.broadcast_to(bo[sl].reshape(1, CH), (128, CH))),
            "sel2": _sel2(),
        })

    kwargs = {}
    if _trace:
        kwargs.update(trace=True,
                      trace_cores=_trace_cores or list(range(N_CORES)))
    res = run_bass_kernel_spmd(nc, in_maps, core_ids=list(range(N_CORES)),
                               **kwargs)

    full = np.empty((B, L, E), dtype=np.float32)
    for c in range(N_CORES):
        b, g = divmod(c, TP)
        full[b, :, g * CH:(g + 1) * CH] = res.results[c]["out"]

    if _trace:
        kernel.last_exec_ns = res.exec_time_ns
        kernel.last_results = res
    return full
